# revision 20
# baseline (speedup 1.0000x reference)
"""Trainium2 Bass kernel for C2C attention (bf16 streaming, PE-fold reduce).

Computes, for x:(B,C,T)=(32,64,30000) f32:
    desc = mean(x, axis=2)                       # (B,C)
    q = desc*Wq + bq ; k = desc*Wk + bk          # (B,C,D), D=64
    attn = softmax(q @ k^T / sqrt(D))            # (B,C,C)
    out = x + alpha * attn @ x
      == (I + alpha*attn) @ x                    # folded residual

Sharding: pure data parallel over batch, 4 batches per core on 8 cores.
On each core, batches are processed as 2 "pairs"; a pair stacks two
batches on the 128 SBUF partitions and uses a block-diagonal 128x128
stationary matrix (I + alpha*attn_b0 (+) I + alpha*attn_b1)^T so one
matmul pass computes both batches.

Resource plan (the kernel is HBM-bound; rel-err budget 2e-2 >> bf16
rounding ~2e-3, so x streams in/out as bf16 = half the f32 traffic):
  - DMA: single SP HWDGE ring, FIFO:  in0 | in1 | out0 | out1.  out0
    sits behind in1 on the ring, bridging pair1's attention chain.
  - PE: main matmuls + ALL mean-reduces ("fold": accumulating matmuls
    with a bf16 identity stationary compress a seg to [128,500] PSUM;
    a ~0.7us DVE tail-reduce finishes).  Direct DVE/ACT seg reduces
    (~6.4us each, no fast mode) would oversubscribe the copy engines.
  - ACT/DVE: PSUM->SBUF drain copies (~1 elem/cycle, f32-PSUM source
    has no fast mode), split 4:2 per seg to match service rates.
  - pair1's input is loaded as 4x7000 + 2000 cols: the last fold after
    in1 lands costs only ~1.5us, pulling the attention chain (and so
    pair1's first output bytes) earlier.  out1's first seg is emitted
    as group-sized DMAs so output traffic resumes at the first copy.
"""

import os

import numpy as np
import ml_dtypes

import concourse.bass as bass
import concourse.tile as tile
from concourse import bacc, mybir
from concourse.bass_utils import run_bass_kernel_spmd


B, C, T, D = 32, 64, 30000, 64
N_CORES = 8
BPC = B // N_CORES          # batches per core = 4
PAIRS = BPC // 2            # 2
ROWS = BPC * C              # 256 rows of (row, T) per core
CHUNK = 500                 # matmul moving free dim (<=512, fits PSUM bank)
GROUP = 2                   # chunks per PSUM tile (2 banks) -> 1000-col copies
GCOLS = GROUP * CHUNK       # 1000

SEGS0 = [6000] * 5                    # pair0 column split
SEGS1 = [7000, 7000, 7000, 7000, 2000]  # pair1: small last seg -> short fold
NSEG = 5

F32 = mybir.dt.float32
BF16 = mybir.dt.bfloat16
AX = mybir.AxisListType
AF = mybir.ActivationFunctionType

# packed constants layout, one (128, 513) f32 block:
#   [:, 0:128]    identity(128)
#   [:, 128:129]  alpha broadcast
#   [0:2, 129:257]   [Wq/(8T) | Wk/T ; bq/8 | bk]
#   [0:2, 257:385]   qk-matmul rhs init: row0 = 0 (sums placeholder), row1 = 1
#   [:, 385:513]  zeros -> attn scratch (off-diagonal blocks must stay 0)
CONST_COLS = 513


def seg_starts(segs):
    out, acc = [], 0
    for w in segs:
        out.append(acc)
        acc += w
    return out


def build_bass() -> bass.Bass:
    nc = bacc.Bacc()

    x = nc.dram_tensor("x", [ROWS, T], BF16, kind="ExternalInput")
    out = nc.dram_tensor("out", [ROWS, T], BF16, kind="ExternalOutput")
    consts_d = nc.dram_tensor("consts", [128, CONST_COLS], F32,
                              kind="ExternalInput")

    segs = [SEGS0, SEGS1]
    starts = [seg_starts(SEGS0), seg_starts(SEGS1)]

    with tile.TileContext(nc) as tc, \
            tc.tile_pool(name="consts", bufs=1) as consts, \
            tc.tile_pool(name="pairbuf", bufs=2) as pairbuf, \
            tc.tile_pool(name="xsegs", bufs=PAIRS * NSEG) as xsegs, \
            tc.tile_pool(name="psmm", bufs=3, space="PSUM") as psmm, \
            tc.tile_pool(name="psmisc", bufs=2, space="PSUM") as psmisc:

        cblk = consts.tile([128, CONST_COLS], F32)
        ident = cblk[:, 0:128]
        alpha_bc = cblk[:, 128:129]
        wqk2 = cblk[0:2, 129:257]
        rhs_qk = cblk[0:2, 257:385]
        attn = cblk[:, 385:513]
        # pre-warm the ACT exp table off the critical path (dummy input)
        scratch = consts.tile([128, 1], F32)
        nc.vector.memset(scratch, 0.0)
        nc.scalar.activation(out=scratch, in_=scratch, func=AF.Exp)
        # bf16 identity, the fold stationary
        identb = consts.tile([128, 128], BF16)

        xs = [[None] * NSEG for _ in range(PAIRS)]
        partials = [None] * PAIRS
        lhsT = [None] * PAIRS

        def emit_in(p):
            part = pairbuf.tile([128, NSEG], F32, tag="partial")
            partials[p] = part
            for s in range(NSEG):
                w = segs[p][s]
                lo = starts[p][s]
                xt = xsegs.tile([128, w], BF16, tag="xseg")
                xs[p][s] = xt
                nc.sync.dma_start(
                    out=xt, in_=x[p * 128:(p + 1) * 128, lo:lo + w])

        def emit_fold(p, s):
            # PE reduces seg s over T: accumulating identity-matmuls fold
            # [128,w] -> PSUM [128,500]; DVE finishes into partials[:, s]
            fp = psmisc.tile([128, 512], F32, tag="fold")
            xt = xs[p][s]
            nchunk = segs[p][s] // CHUNK
            for c in range(nchunk):
                nc.tensor.matmul(
                    out=fp[:, 0:CHUNK], lhsT=identb,
                    rhs=xt[:, c * CHUNK:(c + 1) * CHUNK],
                    start=(c == 0), stop=(c == nchunk - 1),
                )
            nc.vector.reduce_sum(out=partials[p][:, s:s + 1],
                                 in_=fp[:, 0:CHUNK], axis=AX.X)


        def emit_smalls(p):
            # sums over T for both batches of the pair: (128,1) f32
            sums = pairbuf.tile([128, 1], F32, tag="sums")
            nc.vector.reduce_sum(out=sums, in_=partials[p], axis=AX.X)
            # transpose to a row: (1,128)
            srow_ps = psmisc.tile([1, 128], F32, tag="fold")
            nc.tensor.transpose(out=srow_ps, in_=sums, identity=ident)
            nc.vector.tensor_copy(out=rhs_qk[0:1, :], in_=srow_ps)
            # qT/kT = [w; b]^T @ [sums_row; ones] : (D, 2C) covering both batches
            qT_ps = psmisc.tile([D, 2 * C], F32, tag="fold")
            nc.tensor.matmul(out=qT_ps, lhsT=wqk2[:, 0:D], rhs=rhs_qk,
                             start=True, stop=True)
            qT = pairbuf.tile([D, 2 * C], F32, tag="qT")
            nc.vector.tensor_copy(out=qT, in_=qT_ps)
            kT_ps = psmisc.tile([D, 2 * C], F32, tag="fold")
            nc.tensor.matmul(out=kT_ps, lhsT=wqk2[:, D:2 * D], rhs=rhs_qk,
                             start=True, stop=True)
            kT = pairbuf.tile([D, 2 * C], F32, tag="kT")
            nc.vector.tensor_copy(out=kT, in_=kT_ps)
            # logits for both batches on the diagonal blocks of (128,128)
            lg_ps = psmisc.tile([128, 128], F32, tag="fold")
            nc.tensor.matmul(out=lg_ps, lhsT=qT, rhs=kT, start=True, stop=True)
            # exp of each diagonal block; accum_out gives the softmax denom
            sumexp = pairbuf.tile([128, 1], F32, tag="sumexp")
            for h in range(2):
                r = slice(h * 64, h * 64 + 64)
                nc.scalar.activation(
                    out=attn[r, r], in_=lg_ps[r, r], func=AF.Exp,
                    accum_out=sumexp[r, :],
                )
            recip = pairbuf.tile([128, 1], F32, tag="recip")
            nc.vector.reciprocal(out=recip, in_=sumexp)
            nc.vector.tensor_scalar(out=attn, in0=attn, scalar1=recip,
                                    scalar2=alpha_bc,
                                    op0=mybir.AluOpType.mult,
                                    op1=mybir.AluOpType.mult)
            # lhsT = (I + alpha*attn)^T = I + (alpha*attn)^T, cast to bf16
            at_ps = psmisc.tile([128, 128], F32, tag="fold")
            nc.tensor.transpose(out=at_ps, in_=attn, identity=ident)
            ltr = pairbuf.tile([128, 128], BF16, tag="lhsT")
            nc.vector.tensor_add(out=ltr, in0=at_ps, in1=ident)
            lhsT[p] = ltr

        def emit_compute_seg(p, s):
            xt = xs[p][s]
            ngrp = segs[p][s] // GCOLS
            for g in range(ngrp):
                mm = psmm.tile([128, GROUP, 512], F32, tag="mm")
                base = g * GCOLS
                for j in range(GROUP):
                    nc.tensor.matmul(
                        out=mm[:, j, 0:CHUNK],
                        lhsT=lhsT[p],
                        rhs=xt[:, base + j * CHUNK: base + (j + 1) * CHUNK],
                        start=True, stop=True,
                    )
                dst = xt[:, base: base + GCOLS].rearrange(
                    "p (a c) -> p a c", a=GROUP)
                # drain split 4:2 ACT:DVE to match engine service rates
                if g % 3 == 2:
                    nc.vector.tensor_copy(out=dst, in_=mm[:, :, 0:CHUNK])
                else:
                    nc.scalar.copy(out=dst, in_=mm[:, :, 0:CHUNK])

        def emit_out(p, split=()):
            orow = slice(p * 128, (p + 1) * 128)
            for s in range(NSEG):
                w = segs[p][s]
                lo = starts[p][s]
                if s in split:
                    # group-sized transfers: DMA restarts at the first copy
                    for g in range(w // GCOLS):
                        glo = lo + g * GCOLS
                        nc.sync.dma_start(
                            out=out[orow, glo:glo + GCOLS],
                            in_=xs[p][s][:, g * GCOLS:(g + 1) * GCOLS],
                        )
                else:
                    nc.sync.dma_start(out=out[orow, lo:lo + w],
                                      in_=xs[p][s][:, :])

        # --- emission schedule ---
        emit_in(0)
        nc.gpsimd.dma_start(out=cblk, in_=consts_d[:, :])
        nc.scalar.copy(out=identb, in_=ident)
        emit_in(1)                       # ring: in1 right behind in0
        for s in range(NSEG):
            emit_fold(0, s)
        emit_smalls(0)
        for s in range(NSEG):            # fold1 fills PE gaps in main0;
            emit_compute_seg(0, s)       # main0 first: out0 segs are the
            emit_fold(1, s)              # DMA's next deadline, fold1 has slack
        emit_out(0)
        emit_smalls(1)
        for s in range(NSEG):
            emit_compute_seg(1, s)
        emit_out(1, split=(0, NSEG - 1))

    nc.compile()
    return nc


def _host_inputs(x, Wq, bq, Wk, bk, Wv, bv, alpha):
    """Build per-core input maps. Scale folding:
    logits[c,e] = (q[c]/8) . k[e],  q/8 = (Wq/(8T))*sums + bq/8, k = (Wk/T)*sums + bk
    """
    xb = np.asarray(x, dtype=np.float32).astype(ml_dtypes.bfloat16)
    cb = np.zeros((128, CONST_COLS), dtype=np.float32)
    cb[:, 0:128] = np.eye(128, dtype=np.float32)
    cb[:, 128] = np.float32(alpha)
    cb[0, 129:193] = np.asarray(Wq)[:, 0] / (8.0 * T)
    cb[1, 129:193] = np.asarray(bq) / 8.0
    cb[0, 193:257] = np.asarray(Wk)[:, 0] / T
    cb[1, 193:257] = np.asarray(bk)
    cb[1, 257:385] = 1.0
    in_maps = []
    for c in range(N_CORES):
        shard = xb[c * BPC:(c + 1) * BPC].reshape(ROWS, T)
        in_maps.append({
            "x": np.ascontiguousarray(shard),
            "consts": cb,
        })
    return in_maps


def run(inputs: dict, trace: bool = False, tmpdir: str | None = None):
    nc = build_bass()
    in_maps = _host_inputs(**inputs)
    res = run_bass_kernel_spmd(
        nc, in_maps, core_ids=list(range(N_CORES)), trace=trace, tmpdir=tmpdir,
    )
    outs = [np.asarray(m["out"]).astype(np.float32).reshape(BPC, C, T)
            for m in res.results]
    full = np.concatenate(outs, axis=0)
    return full, res


def kernel(**inputs) -> np.ndarray:
    full, _ = run(inputs, trace=bool(os.environ.get("C2C_TRACE")))
    return full


if __name__ == "__main__":
    # quick single-core numerical check in CoreSim
    from concourse import bass_interp

    rng = np.random.default_rng(0)
    x = rng.standard_normal((BPC, C, T), dtype=np.float32)
    Wq = rng.standard_normal((D, 1)).astype(np.float32)
    bq = rng.standard_normal((D,)).astype(np.float32)
    Wk = rng.standard_normal((D, 1)).astype(np.float32)
    bk = rng.standard_normal((D,)).astype(np.float32)
    alpha = np.float32(0.5)

    nc = build_bass()
    sim = bass_interp.CoreSim(nc)
    im = _host_inputs(x=np.tile(x, (N_CORES, 1, 1)), Wq=Wq, bq=bq, Wk=Wk, bk=bk,
                      Wv=None, bv=None, alpha=alpha)[0]
    for k, v in im.items():
        sim.tensor(k)[:] = v
    sim.simulate()
    got = np.asarray(sim.tensor("out")).astype(np.float32).reshape(BPC, C, T)

    desc = x.mean(axis=2, keepdims=True)
    q = desc * Wq[:, 0] + bq
    k = desc * Wk[:, 0] + bk
    logits = np.einsum('bcd,bed->bce', q, k) / np.sqrt(D)
    m = logits.max(axis=-1, keepdims=True)
    e = np.exp(logits - m)
    attn = e / e.sum(axis=-1, keepdims=True)
    mixed = np.einsum('bce,bet->bct', attn, x)
    want = x + alpha * mixed
    err = np.abs(got - want)
    rel = np.linalg.norm(got - want) / np.linalg.norm(want)
    print("max abs err:", err.max(), "rel:", rel)
